# revision 2
# baseline (speedup 1.0000x reference)
"""GCN message-passing kernel for 8 Trainium2 NeuronCores (Bass/Tile).

Strategy:
  - Nodes sharded contiguously across 8 cores (dst-sharded edges, sorted by dst).
  - Per layer: each core computes xl = h @ W + (b+root) for its node shard,
    AllGather's xl to every core, then processes its edges: indirect-DMA
    gather of xl[src] rows, edge-embedding via tiny one-hot matmuls,
    msg = relu(xl[src]+ee)*norm via activation, scatter-add via one-hot
    matmuls into PSUM per 128-node window.
  - BatchNorm folded into per-feature affine applied post-transpose
    (feature-major) on the scalar engine; next layer's xl matmul fused into
    the window loop so node state h never round-trips DRAM.
  - Final layer: head matmul + global-add-pool via one-hot matmuls; per-core
    partial pooled blocks are combined on the host (pure unshard/scatter-add);
    BN shift term and head bias are per-graph affine constants applied on host.
"""

import math
import numpy as np

import concourse.bass as bass
import concourse.bacc as bacc
import concourse.tile as tile
from concourse import mybir
from concourse.bass import IndirectOffsetOnAxis
from concourse.bass_utils import run_bass_kernel_spmd

F32 = mybir.dt.float32
BF16 = mybir.dt.bfloat16
I32 = mybir.dt.int32
BF16_NP = mybir.dt.np(BF16)

AF = mybir.ActivationFunctionType
ALU = mybir.AluOpType

# ----- problem constants (hardcoded; must match reference.py) -----
N_NODES = 200000
N_EDGES = 600000
N_GRAPHS = 4000
EMB = 128
LAYERS = 5
TASKS = 128
ATOM_FEATS, ATOM_VOCAB = 9, 64
BOND_FEATS, BOND_VOCAB = 3, 8
BN_EPS = 1e-5
N_CORES = 8
P = 128  # partitions / tile edge


def _ceil_to(x, m):
    return (x + m - 1) // m * m


class Plan:
    """Host-side preprocessing: sharding, edge tiling, stream layouts."""

    def __init__(self, inputs, n_cores=N_CORES, n_nodes=N_NODES,
                 n_graphs=N_GRAPHS, kg=32):
        self.kg = kg
        self.n_cores = n_cores
        x = np.asarray(inputs["x"])
        edge_index = np.asarray(inputs["edge_index"])
        edge_attr = np.asarray(inputs["edge_attr"])
        batch = np.asarray(inputs["batch"])
        n = n_nodes
        self.n = n
        assert n % n_cores == 0
        sh = n // n_cores
        self.sh = sh
        self.nw = _ceil_to(sh, P) // P          # node windows per core
        self.npad = self.nw * P                  # padded shard size

        src = edge_index[0].astype(np.int64)
        dst = edge_index[1].astype(np.int64)

        # degree (src occurrences + 1), f32 to match reference
        deg = (np.bincount(src, minlength=n).astype(np.float32) + 1.0)
        dinv_sqrt = deg.astype(np.float32) ** -0.5
        norm_e = (dinv_sqrt[src] * dinv_sqrt[dst]).astype(np.float32)
        self.deg_inv = (1.0 / deg).astype(np.float32)

        # sort edges by dst
        order = np.argsort(dst, kind="stable")
        src_s, dst_s, norm_s = src[order], dst[order], norm_e[order]
        attr_s = edge_attr[order].astype(np.int64)

        # per (core, window) edge counts -> shared K_w structure
        core_of = dst_s // sh
        win_of = (dst_s % sh) // P
        counts = np.zeros((n_cores, self.nw), dtype=np.int64)
        np.add.at(counts, (core_of, win_of), 1)
        K_w = np.ceil(counts.max(axis=0) / P).astype(np.int64)
        K_w = np.maximum(K_w, 1)
        self.K_w = K_w.tolist()
        T = int(K_w.sum())
        self.T = _ceil_to(T, kg)
        self.pad_tiles = self.T - T  # trailing all-pad tiles

        # gathered-buffer position of node i: shard r occupies rows
        # [r*npad, r*npad+sh)
        def gpos(nodes):
            return ((nodes // sh) * self.npad + (nodes % sh)).astype(np.int32)

        # build per-core padded edge streams
        E_pad = self.T * P
        self.src_pos = np.zeros((n_cores, P, self.T), dtype=np.int32)
        self.norm_st = np.zeros((n_cores, P, self.T), dtype=np.float32)
        self.dstl_st = np.full((n_cores, P, self.T), -1.0, dtype=np.float32)
        self.oh24 = np.zeros((n_cores, 24, self.T * P), dtype=BF16_NP)

        # tile t -> window map (same for all cores)
        self.tile_win = []
        for w, k in enumerate(self.K_w):
            self.tile_win += [w] * int(k)
        self.tile_win += [None] * self.pad_tiles  # trailing pad tiles

        for c in range(n_cores):
            m = core_of == c
            e_src, e_dst, e_nrm, e_att = src_s[m], dst_s[m], norm_s[m], attr_s[m]
            e_win = (e_dst % sh) // P
            # layout edges into padded stream: window w occupies K_w[w]*P slots
            stream_src = np.zeros(E_pad, dtype=np.int32)
            stream_nrm = np.zeros(E_pad, dtype=np.float32)
            stream_dstl = np.full(E_pad, -1.0, dtype=np.float32)
            stream_code = np.full(E_pad, -1, dtype=np.int64)  # -1 -> zero onehot
            base = 0
            for w, k in enumerate(self.K_w):
                sel = e_win == w
                cnt = int(sel.sum())
                assert cnt <= k * P, (c, w, cnt, k * P)
                sl = slice(base, base + cnt)
                stream_src[sl] = gpos(e_src[sel])
                stream_nrm[sl] = e_nrm[sel]
                stream_dstl[sl] = (e_dst[sel] % sh - w * P).astype(np.float32)
                att = e_att[sel]
                stream_code[sl] = -2  # marker; fill below via per-feature
                # store codes: we need onehot over 24 rows (3 feats x 8 vals)
                for f in range(BOND_FEATS):
                    rows = f * BOND_VOCAB + att[:, f]
                    cols = np.arange(base, base + cnt)
                    self.oh24[c][rows, cols] = 1.0
                base += k * P
            # [P, T] layout: column t holds edges t*P..t*P+P-1 -> reshape(T,P).T
            self.src_pos[c] = stream_src.reshape(self.T, P).T
            self.norm_st[c] = stream_nrm.reshape(self.T, P).T
            self.dstl_st[c] = stream_dstl.reshape(self.T, P).T

        # deg_inv per window column [P, nw]; zero on pad rows
        div = np.zeros((n_cores, P, self.nw), dtype=np.float32)
        for c in range(n_cores):
            d = self.deg_inv[c * sh:(c + 1) * sh]
            d = np.pad(d, (0, self.npad - sh))
            div[c] = d.reshape(self.nw, P).T
        self.dinv_w = div

        # ---- pooling structures ----
        # per node tile: distinct graphs ranked; g_local in [0,128); -1 pad
        self.glocal = np.full((n_cores, P, self.nw), -1.0, dtype=np.float32)
        # host map: graph id per (core, window, slot) for final scatter-add
        self.gmap = np.zeros((n_cores, self.nw, P), dtype=np.int64)
        self.gmap_n = np.zeros((n_cores, self.nw), dtype=np.int64)
        for c in range(n_cores):
            b = batch[c * sh:(c + 1) * sh]
            for w in range(self.nw):
                bw = b[w * P:(w + 1) * P]
                if len(bw) == 0:
                    continue
                uniq, inv = np.unique(np.asarray(bw), return_inverse=True)
                assert len(uniq) <= P
                gl = np.full(P, -1.0, dtype=np.float32)
                gl[:len(bw)] = inv.astype(np.float32)
                self.glocal[c, :, w] = gl
                self.gmap[c, w, :len(uniq)] = uniq
                self.gmap_n[c, w] = len(uniq)

        # per-graph node counts (for host-side BN-shift + bias fixup)
        self.cnt_g = np.bincount(np.asarray(batch), minlength=n_graphs
                                 ).astype(np.float32)

        # atom-encoder one-hot (transposed): [nw, code_chunks* ... ] per core
        # code space: 9 feats x 64 vals = 576 -> pad to 640 (5 chunks of 128)
        self.acodes = ATOM_FEATS * ATOM_VOCAB
        self.achunks = _ceil_to(self.acodes, P) // P
        ohat = np.zeros((n_cores, self.nw, P, self.achunks * P), dtype=BF16_NP)
        for c in range(n_cores):
            xs = x[c * sh:(c + 1) * sh]
            for w in range(self.nw):
                xw = xs[w * P:(w + 1) * P]
                nn = len(xw)
                if nn == 0:
                    continue
                codes = (np.arange(ATOM_FEATS) * ATOM_VOCAB)[None, :] + xw
                # block[code, node] = 1; stored as [node?? -> [P(code_in),
                # chunk, node]] laid out as [code_in P, chunk*P + node]
                oh = np.zeros((self.achunks * P, P), dtype=BF16_NP)
                rows = codes.reshape(nn, ATOM_FEATS)
                for i in range(nn):
                    oh[rows[i], i] = 1.0
                # [code, node] -> chunks: [chunk, 128code, node] -> want
                # [128code, chunk, node]
                ohc = oh.reshape(self.achunks, P, P).transpose(1, 0, 2)
                ohat[c, w] = ohc.reshape(P, self.achunks * P)
        self.oh_atom = ohat

    def weight_arrays(self, inputs):
        """Model-weight-derived arrays (shared across cores)."""
        atom_emb = np.asarray(inputs["atom_emb"], np.float32)  # [9,64,EMB]
        bond_emb = np.asarray(inputs["bond_emb"], np.float32)  # [L,3,8,EMB]
        W = np.asarray(inputs["W"], np.float32)        # [L,EMB,EMB]
        b = np.asarray(inputs["b"], np.float32)        # [L,EMB]
        root = np.asarray(inputs["root"], np.float32)  # [L,EMB]
        bn_mean = np.asarray(inputs["bn_mean"], np.float32)
        bn_var = np.asarray(inputs["bn_var"], np.float32)
        bn_gamma = np.asarray(inputs["bn_gamma"], np.float32)
        bn_beta = np.asarray(inputs["bn_beta"], np.float32)
        headW = np.asarray(inputs["headW"], np.float32)  # [EMB,TASKS]
        headb = np.asarray(inputs["headb"], np.float32)  # [TASKS]

        out = {}
        # atom table: [achunks*P, EMB] padded -> SBUF layout [P, achunks*EMB]
        atab = np.zeros((self.achunks * P, EMB), dtype=np.float32)
        atab[:self.acodes] = atom_emb.reshape(self.acodes, EMB)
        out["atab"] = atab.reshape(self.achunks, P, EMB).transpose(1, 0, 2) \
            .reshape(P, self.achunks * EMB).astype(BF16_NP)
        # W: rhs layout [k_part, (layer, f)]
        out["Wl"] = W.transpose(1, 0, 2).reshape(EMB, LAYERS * EMB).copy()
        out["rootb"] = (root + b).reshape(1, LAYERS * EMB).copy()
        # ee tables: bond sum minus root/3 so msg = relu(xl' + ee')
        eetab = np.zeros((LAYERS, 24, EMB), dtype=np.float32)
        for l in range(LAYERS):
            eetab[l] = bond_emb[l].reshape(24, EMB) + \
                (b[l] - (root[l] + b[l]))[None, :] / BOND_FEATS
        out["eetab"] = eetab.transpose(1, 0, 2).reshape(24, LAYERS * EMB) \
            .astype(BF16_NP)
        # BN fold: scale s, shift t (applied feature-major as [EMB, L] cols)
        s = (bn_gamma / np.sqrt(bn_var + BN_EPS)).astype(np.float32)
        t = (bn_beta - bn_mean * s).astype(np.float32)
        out["bnS"] = s.T.copy()  # [EMB, L]
        out["bnB"] = t.T.copy()
        # head with last-layer BN scale folded
        out["headWp"] = (s[LAYERS - 1][:, None] * headW).astype(np.float32)
        # host-side per-graph constant: cnt*(t4 @ headW) + headb
        self.crow = (t[LAYERS - 1] @ headW).astype(np.float32)
        self.headb = headb
        out["iden"] = np.eye(P, dtype=np.float32)
        out["iota"] = np.tile(np.arange(P, dtype=np.float32), (P, 1))
        out["ones1"] = np.ones((1, P), dtype=np.float32)
        return out

    def postprocess(self, pooled_blocks):
        """pooled_blocks: list per core of [nw, P, TASKS] f32 -> [G, TASKS]."""
        out = np.zeros((N_GRAPHS, TASKS), dtype=np.float32)
        for c in range(self.n_cores):
            blk = pooled_blocks[c]
            for w in range(self.nw):
                k = int(self.gmap_n[c, w])
                if k:
                    np.add.at(out, self.gmap[c, w, :k], blk[w, :k])
        out += self.cnt_g[:, None] * self.crow[None, :] + self.headb[None, :]
        return out


def build_program(plan, debug_stop=None):
    nc = bacc.Bacc(None, target_bir_lowering=False, debug=False)
    nw, T, kg = plan.nw, plan.T, plan.kg
    npad = plan.npad
    ach = plan.achunks
    n_cores = plan.n_cores

    # ---- parameters ----
    def par(name, shape, dt):
        return nc.declare_dram_parameter(name, list(shape), dt, isOutput=False)

    p_ohatom = par("oh_atom", (nw, P, ach * P), BF16)
    p_atab = par("atab", (P, ach * EMB), BF16)
    p_W = par("Wl", (EMB, LAYERS * EMB), F32)
    p_rootb = par("rootb", (1, LAYERS * EMB), F32)
    p_eetab = par("eetab", (24, LAYERS * EMB), BF16)
    p_bnS = par("bnS", (EMB, LAYERS), F32)
    p_bnB = par("bnB", (EMB, LAYERS), F32)
    p_headW = par("headWp", (EMB, TASKS), F32)
    p_iden = par("iden", (P, P), F32)
    p_iota = par("iota", (P, P), F32)
    p_ones1 = par("ones1", (1, P), F32)
    p_src = par("src_pos", (P, T), I32)
    p_norm = par("norm_st", (P, T), F32)
    p_dstl = par("dstl_st", (P, T), F32)
    p_oh24 = par("oh24", (24, T * P), BF16)
    p_dinv = par("dinv_w", (P, nw), F32)
    p_gloc = par("glocal", (P, nw), F32)
    p_out = nc.declare_dram_parameter("out", [nw, P, TASKS], F32, isOutput=True)
    p_dbg = None
    if debug_stop is not None:
        p_dbg = nc.declare_dram_parameter("dbg", [nw * P, EMB], F32,
                                          isOutput=True)

    # ---- internal DRAM ----
    xl_sh = [nc.dram_tensor(f"xl_sh{i}", [npad, EMB], F32) for i in range(2)]
    xl_full = nc.dram_tensor("xl_full", [n_cores * npad, EMB], F32,
                             addr_space="Shared")

    groups = [list(range(n_cores))]
    n_chunks = T // kg

    with tile.TileContext(nc) as tc:
        with tc.tile_pool(name="const", bufs=1) as cpool, \
             tc.tile_pool(name="sb", bufs=2) as sb, \
             tc.tile_pool(name="sb3", bufs=3) as sb3, \
             tc.tile_pool(name="psA", bufs=2, space="PSUM") as psA, \
             tc.tile_pool(name="psE", bufs=2, space="PSUM") as psE, \
             tc.tile_pool(name="psM", bufs=3, space="PSUM") as psM:

            # ---- resident constants / streams ----
            def cload(ap, shape, dt, name):
                t = cpool.tile(list(shape), dt, tag=name)
                nc.sync.dma_start(out=t[:], in_=ap)
                return t

            iden = cload(p_iden[:, :], (P, P), F32, "iden")
            iota = cload(p_iota[:, :], (P, P), F32, "iota")
            ones1 = cload(p_ones1[:, :], (1, P), F32, "ones1")
            atab = cload(p_atab[:, :], (P, ach * EMB), BF16, "atab")
            Wl = cload(p_W[:, :], (EMB, LAYERS * EMB), F32, "Wl")
            rootb = cload(p_rootb[:, :], (1, LAYERS * EMB), F32, "rootb")
            eetab = cload(p_eetab[:, :], (24, LAYERS * EMB), BF16, "eetab")
            bnS = cload(p_bnS[:, :], (EMB, LAYERS), F32, "bnS")
            bnB = cload(p_bnB[:, :], (EMB, LAYERS), F32, "bnB")
            headW = cload(p_headW[:, :], (EMB, TASKS), F32, "headW")
            srcs = cload(p_src[:, :], (P, T), I32, "srcs")
            norms = cload(p_norm[:, :], (P, T), F32, "norms")
            dstls = cload(p_dstl[:, :], (P, T), F32, "dstls")
            dinvw = cload(p_dinv[:, :], (P, nw), F32, "dinvw")
            glocw = cload(p_gloc[:, :], (P, nw), F32, "glocw")

            # ---------------- encoder: xl_0 per window ----------------
            for w in range(nw):
                ohw = sb.tile([P, ach * P], BF16, tag="ohw")
                nc.sync.dma_start(out=ohw[:], in_=p_ohatom[w, :, :])
                h0p = psM.tile([P, EMB], F32, tag="mm")
                for c in range(ach):
                    nc.tensor.matmul(
                        out=h0p[:], lhsT=ohw[:, c * P:(c + 1) * P],
                        rhs=atab[:, c * EMB:(c + 1) * EMB],
                        start=(c == 0), stop=(c == ach - 1))
                h0s = sb.tile([P, EMB], F32, tag="h0s")
                nc.scalar.activation(out=h0s[:], in_=h0p[:], func=AF.Copy)
                hTp = psM.tile([P, EMB], F32, tag="mm")
                nc.tensor.transpose(out=hTp[:], in_=h0s[:], identity=iden[:])
                hTs = sb.tile([P, EMB], F32, tag="hTs")
                nc.scalar.activation(out=hTs[:], in_=hTp[:], func=AF.Copy)
                xlp = psM.tile([P, EMB], F32, tag="mm")
                nc.tensor.matmul(out=xlp[:], lhsT=hTs[:], rhs=Wl[:, 0:EMB],
                                 start=True, stop=False)
                nc.tensor.matmul(out=xlp[:], lhsT=ones1[:],
                                 rhs=rootb[0:1, 0:EMB], start=False, stop=True)
                xls = sb.tile([P, EMB], F32, tag="xls")
                nc.vector.tensor_copy(out=xls[:], in_=xlp[:])
                nc.sync.dma_start(out=xl_sh[0][w * P:(w + 1) * P, :],
                                  in_=xls[:])

            # ---------------- layers ----------------
            n_layers_run = LAYERS if debug_stop is None else debug_stop
            for l in range(n_layers_run):
                cur, nxt = xl_sh[l % 2], xl_sh[(l + 1) % 2]
                nc.gpsimd.collective_compute(
                    "AllGather", ALU.bypass,
                    ins=[cur[:, :].opt()], outs=[xl_full[:, :].opt()],
                    replica_groups=groups)

                # chunk machinery state
                gbuf = sel = msgb = oh24c = None
                t_idx = 0

                def emit_chunk(j):
                    nonlocal gbuf, sel, msgb, oh24c
                    t0 = j * kg
                    gbuf = sb.tile([P, kg * P], F32, tag="gbuf")
                    for i in range(kg):
                        nc.gpsimd.indirect_dma_start(
                            out=gbuf[:, i * P:(i + 1) * P], out_offset=None,
                            in_=xl_full[:, :],
                            in_offset=IndirectOffsetOnAxis(
                                ap=srcs[:, t0 + i:t0 + i + 1], axis=0))
                    sel = sb.tile([P, kg * P], BF16, tag="sel")
                    nc.vector.tensor_tensor(
                        out=sel[:].rearrange("p (k e) -> p k e", k=kg),
                        in0=dstls[:, t0:t0 + kg].unsqueeze(2)
                            .to_broadcast([P, kg, P]),
                        in1=iota[:].unsqueeze(1).to_broadcast([P, kg, P]),
                        op=ALU.is_equal)
                    oh24c = sb3.tile([24, kg * P], BF16, tag="oh24c")
                    nc.sync.dma_start(out=oh24c[:],
                                      in_=p_oh24[:, t0 * P:(t0 + kg) * P])
                    msgb = sb.tile([P, kg * P], BF16, tag="msgb")

                def emit_group(g0):
                    # g0: global tile index of 4-tile group start
                    j = g0 // kg
                    base = (g0 % kg) * P
                    ng = min(4, T - g0)
                    eep = psE.tile([P, 4 * P], F32, tag="ee")
                    for i in range(ng):
                        nc.tensor.matmul(
                            out=eep[:, i * P:(i + 1) * P],
                            lhsT=oh24c[:, base + i * P: base + (i + 1) * P],
                            rhs=eetab[:, l * EMB:(l + 1) * EMB],
                            start=True, stop=True)
                    # msg_pre = gathered + ee  (write bf16)
                    nc.vector.tensor_tensor(
                        out=msgb[:, base:base + ng * P],
                        in0=gbuf[:, base:base + ng * P],
                        in1=eep[:, :ng * P], op=ALU.add)
                    return eep

                # window loop
                for w in range(nw):
                    aggp = psA.tile([P, EMB], F32, tag="agg")
                    kw = plan.K_w[w]
                    for i in range(kw):
                        t = t_idx + i
                        if t % kg == 0:
                            emit_chunk(t // kg)
                        if t % 4 == 0:
                            emit_group(t)
                        base = (t % kg) * P
                        # msg = relu(msg_pre) * norm  (norm>0)
                        nc.scalar.activation(
                            out=msgb[:, base:base + P],
                            in_=msgb[:, base:base + P], func=AF.Relu,
                            scale=norms[:, t:t + 1])
                        nc.tensor.matmul(
                            out=aggp[:], lhsT=sel[:, base:base + P],
                            rhs=msgb[:, base:base + P],
                            start=(i == 0), stop=(i == kw - 1))
                    t_idx += kw

                    # self term + combine
                    xlo = sb.tile([P, EMB], F32, tag="xlo")
                    nc.sync.dma_start(out=xlo[:],
                                      in_=cur[w * P:(w + 1) * P, :])
                    sf = sb.tile([P, EMB], F32, tag="sf")
                    nc.scalar.activation(out=sf[:], in_=xlo[:], func=AF.Relu,
                                         scale=dinvw[:, w:w + 1])
                    hnew = sb.tile([P, EMB], F32, tag="hnew")
                    nc.vector.tensor_tensor(out=hnew[:], in0=sf[:],
                                            in1=aggp[:], op=ALU.add)
                    hTp = psM.tile([P, EMB], F32, tag="mm")
                    nc.tensor.transpose(out=hTp[:], in_=hnew[:],
                                        identity=iden[:])
                    hTs = sb.tile([P, EMB], F32, tag="hTs")
                    if l < LAYERS - 1:
                        # BN affine + relu (feature-major: per-partition)
                        nc.scalar.activation(
                            out=hTs[:], in_=hTp[:], func=AF.Relu,
                            scale=bnS[:, l:l + 1], bias=bnB[:, l:l + 1])
                        xlp = psM.tile([P, EMB], F32, tag="mm")
                        nc.tensor.matmul(out=xlp[:], lhsT=hTs[:],
                                         rhs=Wl[:, (l + 1) * EMB:
                                                (l + 2) * EMB],
                                         start=True, stop=False)
                        nc.tensor.matmul(out=xlp[:], lhsT=ones1[:],
                                         rhs=rootb[0:1, (l + 1) * EMB:
                                                  (l + 2) * EMB],
                                         start=False, stop=True)
                        xls = sb.tile([P, EMB], F32, tag="xls")
                        nc.vector.tensor_copy(out=xls[:], in_=xlp[:])
                        nc.sync.dma_start(out=nxt[w * P:(w + 1) * P, :],
                                          in_=xls[:])
                    else:
                        # last layer: no BN here (folded into head); head+pool
                        nc.scalar.activation(out=hTs[:], in_=hTp[:],
                                             func=AF.Copy)
                        zp = psM.tile([P, TASKS], F32, tag="mm")
                        nc.tensor.matmul(out=zp[:], lhsT=hTs[:],
                                         rhs=headW[:], start=True, stop=True)
                        zs = sb.tile([P, TASKS], F32, tag="zs")
                        nc.scalar.activation(out=zs[:], in_=zp[:],
                                             func=AF.Copy)
                        selg = sb.tile([P, P], F32, tag="selg")
                        nc.vector.tensor_tensor(
                            out=selg[:],
                            in0=glocw[:, w:w + 1].to_broadcast([P, P]),
                            in1=iota[:], op=ALU.is_equal)
                        pp = psM.tile([P, TASKS], F32, tag="mm")
                        nc.tensor.matmul(out=pp[:], lhsT=selg[:], rhs=zs[:],
                                         start=True, stop=True)
                        ps = sb.tile([P, TASKS], F32, tag="ps")
                        nc.vector.tensor_copy(out=ps[:], in_=pp[:])
                        nc.sync.dma_start(out=p_out[w, :, :], in_=ps[:])

            if debug_stop is not None:
                # dump xl state entering layer `debug_stop` (SBUF roundtrip)
                for w in range(nw):
                    dt_ = sb.tile([P, EMB], F32, tag="dbgt")
                    nc.sync.dma_start(
                        out=dt_[:],
                        in_=xl_sh[debug_stop % 2][w * P:(w + 1) * P, :])
                    nc.sync.dma_start(out=p_dbg[w * P:(w + 1) * P, :],
                                      in_=dt_[:])

    nc.finalize()
    return nc


_CACHE = {}


def kernel(**inputs):
    key = "prog"
    if key not in _CACHE:
        plan = Plan(inputs)
        warr = plan.weight_arrays(inputs)
        nc = build_program(plan)
        _CACHE[key] = (plan, nc)
    else:
        plan, nc = _CACHE[key]
        warr = plan.weight_arrays(inputs)

    in_maps = []
    for c in range(plan.n_cores):
        m = dict(warr)
        m["oh_atom"] = plan.oh_atom[c]
        m["src_pos"] = plan.src_pos[c]
        m["norm_st"] = plan.norm_st[c]
        m["dstl_st"] = plan.dstl_st[c]
        m["oh24"] = plan.oh24[c]
        m["dinv_w"] = plan.dinv_w[c]
        m["glocal"] = plan.glocal[c]
        in_maps.append(m)

    import os
    trace = bool(os.environ.get("BASS_GNN_TRACE"))
    if trace:
        try:
            import ntff_hook
            ntff_hook.install()
        except Exception:
            trace = False
    res = run_bass_kernel_spmd(nc, in_maps, list(range(plan.n_cores)),
                               trace=trace)
    global _LAST_EXEC_NS, _LAST_RES
    _LAST_EXEC_NS = res.exec_time_ns
    _LAST_RES = res
    blocks = [np.asarray(r["out"], np.float32) for r in res.results]
    return plan.postprocess(blocks)

